# revision 22
# baseline (speedup 1.0000x reference)
"""Causal self-attention (GPT-2 small block shape: B=4, T=2048, C=768, H=12, D=64)
on 8 TRN2 NeuronCores.

Sharding: core i handles batch b = i//2 and head-half = i%2 (6 heads each).
No cross-core collectives; the two half-head partial output projections per
batch are summed on the host during unshard (row-parallel c_proj).

Device kernel (per core, all matmuls bf16, fp32 PSUM accumulation):
  1. qkv^T = w^T x^T via TensorE with contraction over C (x^T supplied
     pre-transposed + bf16 by the host).  Q is pre-scaled by 1/sqrt(D) and
     Q/K biases are added on evacuation (V bias is folded into b_proj on
     the host since softmax rows sum to 1).
  2. Per head pair: S^T[k,q] = K^T.T @ Q^T blocks with the two heads'
     K=64 matmuls issued back-to-back at PE row-tiles (0,0)/(64,0) so they
     run concurrently; exp on ScalarE straight out of PSUM with the
     leading causally-dead columns trimmed; diagonal-block masking on DVE.
  3. AV with V augmented by a ones column -> row sums land in PSUM
     partition 64 for free; normalization = reciprocal_approx_fast (DVE,
     straight from PSUM) + gpsimd broadcast + one tensor-tensor multiply.
  4. Output projection computed transposed (out^T[c,t], contraction over
     the head dim) so b_proj lands as a per-partition tensor_scalar add;
     host re-transposes.  QKV/proj tensor work is interleaved into the
     ScalarE-bound attention rounds as PE filler.
"""

import sys

if "/opt/trn_rl_repo" not in sys.path:
    sys.path.insert(0, "/opt/trn_rl_repo")

import numpy as np
import ml_dtypes

import concourse.bass as bass  # noqa: F401  (engine types pulled via nc)
import concourse.mybir as mybir
from concourse import bacc
from concourse.tile import TileContext
from concourse.bass_utils import run_bass_kernel_spmd

BF16 = ml_dtypes.bfloat16

B, T, C = 4, 2048, 768
H, D = 12, 64
NH = 6  # heads per core
P = 128
TC = T // P  # 16 t-chunks of 128
QC = T // 512  # 4 q-chunks of 512
CCH = C // P  # 6 contraction chunks

DT = mybir.dt.bfloat16
F32 = mybir.dt.float32


def build_nc():
    nc = bacc.Bacc()

    xt_d = nc.declare_dram_parameter("xt", [P, CCH, T], DT, isOutput=False)
    wqk_d = nc.declare_dram_parameter("wqk", [P, CCH, 2 * NH * D], DT, isOutput=False)
    bqk_d = nc.declare_dram_parameter("bqk", [P, 2 * NH * D // P], F32, isOutput=False)
    wv_d = nc.declare_dram_parameter("wv", [P, CCH, NH * D], DT, isOutput=False)
    wp_d = nc.declare_dram_parameter(
        "wp", [P, NH * D // P, C // P, P], DT, isOutput=False
    )
    bp_d = nc.declare_dram_parameter("bp", [P, C // P], F32, isOutput=False)
    mask_d = nc.declare_dram_parameter("mask", [P, P], DT, isOutput=False)
    out_d = nc.declare_dram_parameter("out", [P, C // P, T], F32, isOutput=True)

    with TileContext(nc) as tc:
        with (
            tc.tile_pool(name="consts", bufs=1) as consts,
            tc.tile_pool(name="work", bufs=4) as work,
            tc.tile_pool(name="outp", bufs=3) as outp,
            tc.tile_pool(name="ppool", bufs=1) as ppool,
            tc.tile_pool(name="ps_s", bufs=1, space="PSUM") as ps_s,
            tc.tile_pool(name="ps_qkv", bufs=2, space="PSUM") as ps_qkv,
            tc.tile_pool(name="ps_av", bufs=1, space="PSUM") as ps_av,
        ):
            # ---- load inputs ----
            # inputs split across the two HWDGE queues (sync + scalar);
            # round-0 dependencies (xt quarter 0, wqk, wv, bqk) first
            # sync queue: xt quarter 0 first (round-0 moving operand), then wv
            xt_sb = consts.tile([P, CCH, T], DT)
            nc.sync.dma_start(xt_sb[:, :, 0 : T // 4], xt_d[:, :, 0 : T // 4])
            wv_sb = consts.tile([P, CCH, NH * D], DT)
            nc.sync.dma_start(wv_sb[:], wv_d[:])
            # scalar queue: mask (warmup operand) + per-fc wqk slices so the
            # first qk_tiles start ~15us earlier than a monolithic wqk load
            mask_sb = consts.tile([P, P], DT)
            nc.scalar.dma_start(mask_sb[:], mask_d[:])
            wqk_sb = consts.tile([P, CCH, 2 * NH * D], DT)
            for fc in (0, 3):
                nc.scalar.dma_start(
                    wqk_sb[:, :, fc * P : (fc + 1) * P],
                    wqk_d[:, :, fc * P : (fc + 1) * P],
                )
            bqk_sb = consts.tile([P, 2 * NH * D // P], F32)
            nc.scalar.dma_start(bqk_sb[:], bqk_d[:])
            for fc in (1, 4, 2, 5):
                nc.scalar.dma_start(
                    wqk_sb[:, :, fc * P : (fc + 1) * P],
                    wqk_d[:, :, fc * P : (fc + 1) * P],
                )
            for _q in range(1, 4):
                eng = nc.scalar if _q % 2 else nc.sync
                eng.dma_start(
                    xt_sb[:, :, _q * T // 4 : (_q + 1) * T // 4],
                    xt_d[:, :, _q * T // 4 : (_q + 1) * T // 4],
                )
            wp_sb = consts.tile([P, NH * D // P, C // P, P], DT)
            nc.scalar.dma_start(wp_sb[:], wp_d[:])
            bp_sb = consts.tile([P, C // P], F32)
            nc.sync.dma_start(bp_sb[:], bp_d[:])

            # Q^T/K^T as head-pair tiles [128, T]: head 2p in partitions 0:64,
            # head 2p+1 in partitions 64:128 (S-gen uses PE row tiles 0/64)
            qtp = [consts.tile([P, T], DT, name=f"qtp{p}", tag=f"qtp{p}") for p in range(NH // 2)]
            ktp = [consts.tile([P, T], DT, name=f"ktp{p}", tag=f"ktp{p}") for p in range(NH // 2)]
            # V per t-chunk, heads side by side with a ones column: [128, 6, 65]
            vt = [consts.tile([P, NH, D + 1], DT, name=f"vt{t}", tag=f"vt{t}") for t in range(TC)]
            for t in range(TC):
                nc.gpsimd.memset(vt[t][:, :, D : D + 1], 1.0)
            # y^T per head-pair [128, T] bf16
            yt = [consts.tile([P, T], DT, name=f"yt{p}", tag=f"yt{p}") for p in range(NH // 2)]



            # ---- filler units (QKV / V / proj work interleaved into the
            # ScalarE-bound attention rounds to keep the PE busy) ----
            def qk_tile(fc, tcb):
                # feat chunk fc: 0..2 -> Q pair fc, 3..5 -> K pair fc-3
                pq = ps_qkv.tile([P, 512], F32, tag="qkv", name="pq")
                for cc in range(CCH):
                    nc.tensor.matmul(
                        pq[:],
                        wqk_sb[:, cc, fc * P : (fc + 1) * P],
                        xt_sb[:, cc, tcb * 512 : (tcb + 1) * 512],
                        start=(cc == 0),
                        stop=(cc == CCH - 1),
                    )
                dst = qtp[fc] if fc < 3 else ktp[fc - 3]
                nc.vector.tensor_scalar_add(
                    dst[:, tcb * 512 : (tcb + 1) * 512],
                    pq[:],
                    bqk_sb[:, fc : fc + 1],
                )

            def v_chunk(t):
                pv = ps_qkv.tile([P, NH * D], F32, tag="qkv", name="pv")
                for cc in range(CCH):
                    nc.tensor.matmul(
                        pv[:],
                        xt_sb[:, cc, t * P : (t + 1) * P],
                        wv_sb[:, cc, :],
                        start=(cc == 0),
                        stop=(cc == CCH - 1),
                    )
                nc.vector.tensor_copy(
                    vt[t][:, :, 0:D],
                    pv[:].rearrange("p (h d) -> p h d", d=D),
                )

            def proj(cc, qc):
                # out^T[c-chunk cc, t-range qc]: contract over head dim
                pp = ps_qkv.tile([P, 512], F32, tag="qkv", name="pp")
                for hp in range(NH // 2):
                    nc.tensor.matmul(
                        pp[:],
                        wp_sb[:, hp, cc, :],
                        yt[hp][:, qc * 512 : (qc + 1) * 512],
                        start=(hp == 0),
                        stop=(hp == NH // 2 - 1),
                    )
                stg = outp.tile([P, 512], F32)
                nc.vector.tensor_scalar_add(stg[:], pp[:], bp_sb[:, cc : cc + 1])
                nc.sync.dma_start(out_d[:, cc, qc * 512 : (qc + 1) * 512], stg[:])

            # last-round proj is split so the hp 0/1 partials run mid-round
            # and only the hp 2 matmul + finish are left for the tail
            proj_part = {}

            def proj_a(cc, qc):
                pp = ps_qkv.tile([P, 512], F32, tag="qkv", name="ppa")
                for hp in (0, 1):
                    nc.tensor.matmul(
                        pp[:],
                        wp_sb[:, hp, cc, :],
                        yt[hp][:, qc * 512 : (qc + 1) * 512],
                        start=(hp == 0),
                        stop=(hp == 1),
                    )
                part = ppool.tile([P, 512], F32, tag=f"part{cc}", name="part")
                proj_part[cc] = part
                nc.vector.tensor_scalar_add(part[:], pp[:], bp_sb[:, cc : cc + 1])

            def proj_b(cc, qc):
                pp = ps_qkv.tile([P, 512], F32, tag="qkv", name="ppb")
                nc.tensor.matmul(
                    pp[:],
                    wp_sb[:, 2, cc, :],
                    yt[2][:, qc * 512 : (qc + 1) * 512],
                    start=True,
                    stop=True,
                )
                stg = outp.tile([P, 512], F32)
                nc.vector.tensor_add(stg[:], pp[:], proj_part[cc][:])
                nc.sync.dma_start(out_d[:, cc, qc * 512 : (qc + 1) * 512], stg[:])

            # ---- attention: flat software pipeline over (qc, hp, g) with the
            # AV stage lagging one group behind the S+exp stage, so the PE
            # always has ready S-pair work ahead of the exp-gated AV matmuls
            av_tiles = {}

            def s_exp_stage(qc, hp, g):
                sps, sexp = [], []
                for hi in (0, 1):
                    sps.append(ps_s.tile([P, 1024], F32, tag=f"s{hi}", name=f"sps{hi}"))
                # S pair: interleave the two heads' K=64 matmuls so the
                # PE row-tiles (0,0)/(64,0) execute them concurrently
                for sub in range(2):
                    j = 2 * g + sub
                    m = max(0, (j - 4 * qc) * P)  # causal trim offset
                    # keep sub 1 un-trimmed so the exp input range
                    # [m0:1024] is fully written (extra cols never read)
                    mw = m if sub == 0 else 0
                    for hi in (0, 1):
                        b0 = 64 * hi
                        nc.tensor.matmul(
                            sps[hi][:, sub * 512 + mw : (sub + 1) * 512],
                            ktp[hp][b0 : b0 + 64, j * P : (j + 1) * P],
                            qtp[hp][b0 : b0 + 64, qc * 512 + mw : (qc + 1) * 512],
                            start=True,
                            stop=True,
                        )
                m0 = max(0, (2 * g - 4 * qc) * P)  # leading dead cols
                for hi in (0, 1):
                    se = work.tile([P, 1024], DT, tag="sexp", name="sexp")
                    sexp.append(se)
                    nc.scalar.activation(
                        se[:, m0:1024],
                        sps[hi][:, m0:1024],
                        mybir.ActivationFunctionType.Exp,
                    )
                return sexp

            def av_stage(qc, hp, g, sexp):
                nj = 4 * (qc + 1)
                if g == 0:
                    av_tiles[(qc, hp)] = [
                        ps_av.tile([65, 512], F32, tag=f"av{hi}", name=f"av{hi}")
                        for hi in (0, 1)
                    ]
                av = av_tiles[(qc, hp)]
                for hi in (0, 1):
                    for sub in range(2):
                        j = 2 * g + sub
                        m = max(0, (j - 4 * qc) * P)
                        if j - 4 * qc >= 0:  # diagonal block: mask (GpSimd —
                            # keeps the AV matmuls off the busy DVE queue)
                            nc.gpsimd.tensor_mul(
                                sexp[hi][:, sub * 512 + m : sub * 512 + m + P],
                                sexp[hi][:, sub * 512 + m : sub * 512 + m + P],
                                mask_sb[:],
                            )
                        nc.tensor.matmul(
                            av[hi][:, m:512],
                            vt[j][:, 2 * hp + hi, :],
                            sexp[hi][:, sub * 512 + m : (sub + 1) * 512],
                            start=(j == 0),
                            stop=(j == nj - 1),
                            skip_group_check=True,
                        )
                if 2 * g + 1 == nj - 1:  # last group: normalize
                    for hi in (0, 1):
                        b0 = 64 * hi
                        # two copies evacuate the av PSUM slot quickly; the
                        # rest of the norm chain runs from SBUF.  The sums
                        # row is copied to partition 0: the custom-DVE
                        # reciprocal misbehaves on a partition-64 source.
                        yc = work.tile([64, 512], F32, tag="yc", name="yc")
                        nc.vector.tensor_copy(yc[:], av[hi][0:64, :])
                        sums = work.tile([1, 512], F32, tag="sums", name="sums")
                        nc.vector.tensor_copy(sums[:], av[hi][64:65, :])
                        inv = work.tile([1, 512], F32, tag="inv", name="inv")
                        nc.vector.reciprocal_approx_fast(inv[:], sums[:])
                        invb = work.tile([64, 512], F32, tag="invb", name="invb")
                        nc.gpsimd.partition_broadcast(invb[:], inv[:])
                        nc.vector.tensor_mul(
                            yt[hp][b0 : b0 + 64, qc * 512 : (qc + 1) * 512],
                            yc[:],
                            invb[:],
                        )

            # ---- warm the PE HAM clock gate during the input DMA wait ----
            for w in range(24):
                wps = ps_qkv.tile([P, P], F32, tag="qkv", name="warm")
                nc.tensor.matmul(wps[:], mask_sb[:], mask_sb[:], start=True, stop=True)

            # ---- schedule: flat software pipeline over (qc, hp, g) with a
            # deadline-driven filler queue.  Q tiles are due at their round's
            # start; K/V tiles are lazily due at their first (diagonal) use
            # inside their own round; proj has no deadline.  This levels the
            # PE filler load across the growing causal rounds.
            flat = [
                (qc, hp, g)
                for qc in range(QC)
                for hp in range(3)
                for g in range(2 * (qc + 1))
            ]
            fidx = {key: i for i, key in enumerate(flat)}
            END = len(flat) + 1
            sched = []  # (deadline flat-index, closure), FIFO within pushes

            def push(deadline, fn):
                sched.append([deadline, fn])

            def emit_due(i):
                rest = []
                for item in sched:
                    if item[0] <= i:
                        item[1]()
                    else:
                        rest.append(item)
                sched[:] = rest

            # round 0 critical prefix
            qk_tile(0, 0)
            qk_tile(3, 0)
            for fc in (1, 2):
                push(fidx[(0, fc, 0)], lambda fc=fc: qk_tile(fc, 0))
                push(fidx[(0, fc, 0)], lambda fc=fc: qk_tile(3 + fc, 0))
            for t in range(4):
                push(fidx[(0, 0, t // 2)] + 1, lambda t=t: v_chunk(t))

            prev = None
            cur_qc = -1
            periods_left = 0
            for i, (qc, hp, g) in enumerate(flat):
                if qc != cur_qc:
                    cur_qc = qc
                    periods_left = 6 * (qc + 1)
                    if qc + 1 < QC:
                        r = qc + 1
                        for fc in range(3):
                            push(fidx[(r, fc, 0)], lambda fc=fc, r=r: qk_tile(fc, r))
                        for p in range(3):
                            push(
                                fidx[(r, p, 2 * r)],
                                lambda p=p, r=r: qk_tile(3 + p, r),
                            )
                        for t in range(4 * r, 4 * r + 4):
                            push(
                                fidx[(r, 0, t // 2)] + 1,
                                lambda t=t: v_chunk(t),
                            )
                    if qc > 0:
                        for cc in range(C // P):
                            push(END, lambda cc=cc, qc=qc: proj(cc, qc - 1))
                if qc == QC - 1 and hp == 2 and g == 0:
                    # hp 0/1 of the last round are normalized: queue partials
                    for cc in range(C // P):
                        push(END, lambda cc=cc: proj_a(cc, QC - 1))
                emit_due(i)
                sexp = s_exp_stage(qc, hp, g)
                if prev is not None:
                    av_stage(*prev)
                    npop = min(len(sched), 2 if len(sched) > periods_left else 1)
                    for _ in range(npop):
                        sched.pop(0)[1]()
                prev = (qc, hp, g, sexp)
                periods_left -= 1
            av_stage(*prev)
            # keep the PE warm through the final normalization chain
            for w in range(12):
                wps = ps_qkv.tile([P, 512], F32, tag="qkv", name="warm2")
                nc.tensor.matmul(
                    wps[:], mask_sb[:], xt_sb[:, 0, 0:512], start=True, stop=True
                )
            for item in sched:
                item[1]()
            for cc in range(C // P):
                proj_b(cc, QC - 1)

    nc.finalize()
    return nc


def shard_inputs(x, w_attn, b_attn, w_proj, b_proj):
    """Host-side prep: slice per core, transpose x, cast to bf16."""
    scale = 1.0 / np.sqrt(D)
    tril = np.tril(np.ones((P, P), np.float32))
    # mask[k_local, q_local] = 1 where k <= q
    mask = tril.T.astype(BF16)
    in_maps = []
    for core in range(8):
        b, half = divmod(core, 2)
        h0 = half * NH
        cq = slice(h0 * D, (h0 + NH) * D)
        ck = slice(C + h0 * D, C + (h0 + NH) * D)
        cv = slice(2 * C + h0 * D, 2 * C + (h0 + NH) * D)
        wq = (w_attn[:, cq] * scale).astype(BF16)
        wk = w_attn[:, ck].astype(BF16)
        wqk = np.concatenate([wq, wk], axis=1)  # [C, 768]
        bqk = np.concatenate([(b_attn[cq] * scale), b_attn[ck]], axis=0).astype(
            np.float32
        )
        bqk_col = np.ascontiguousarray(bqk.reshape(2 * NH * D // P, P).T)
        wv = w_attn[:, cv].astype(BF16)
        bv = b_attn[cv].astype(np.float32)
        wp = w_proj[h0 * D : (h0 + NH) * D, :].astype(np.float32)
        # V bias rides through softmax (rows sum to 1): fold into proj bias
        bp_eff = bv @ wp + (b_proj if half == 0 else np.zeros_like(b_proj))
        bp_col = np.ascontiguousarray(bp_eff.reshape(C // P, P).T.astype(np.float32))
        xt = np.ascontiguousarray(x[b].T).astype(BF16)  # [C, T]
        in_maps.append(
            {
                "xt": np.ascontiguousarray(
                    xt.reshape(CCH, P, T).transpose(1, 0, 2)
                ),
                "wqk": np.ascontiguousarray(
                    wqk.reshape(CCH, P, 2 * NH * D).transpose(1, 0, 2)
                ),
                "bqk": bqk_col,
                "wv": np.ascontiguousarray(
                    wv.reshape(CCH, P, NH * D).transpose(1, 0, 2)
                ),
                "wp": np.ascontiguousarray(
                    wp.astype(BF16).reshape(NH * D // P, P, C // P, P).transpose(1, 0, 2, 3)
                ),
                "bp": bp_col,
                "mask": mask,
            }
        )
    return in_maps


_NC = None


def _get_nc():
    global _NC
    if _NC is None:
        _NC = build_nc()
    return _NC


def run_sharded(in_maps, trace=False, **kw):
    nc = _get_nc()
    return run_bass_kernel_spmd(nc, in_maps, core_ids=list(range(8)), trace=trace, **kw)


def gather(results):
    out = np.zeros((B, T, C), np.float32)
    for core in range(8):
        b = core // 2
        # out^T [P, C//P, T]: C index = cc*128 + p
        ot = results[core]["out"]
        out[b] += ot.transpose(1, 0, 2).reshape(C, T).T
    return out


def kernel(x, w_attn, b_attn, w_proj, b_proj):
    x = np.asarray(x, np.float32)
    w_attn = np.asarray(w_attn, np.float32)
    b_attn = np.asarray(b_attn, np.float32)
    w_proj = np.asarray(w_proj, np.float32)
    b_proj = np.asarray(b_proj, np.float32)
    in_maps = shard_inputs(x, w_attn, b_attn, w_proj, b_proj)
    res = run_sharded(in_maps, trace=False)
    return gather(res.results)


# revision 23
# speedup vs baseline: 1.7852x; 1.7852x over previous
"""Causal self-attention (GPT-2 small block shape: B=4, T=2048, C=768, H=12, D=64)
on 8 TRN2 NeuronCores.

Sharding: core i handles batch b = i//2 and head-half = i%2 (6 heads each).
No cross-core collectives; the two half-head partial output projections per
batch are summed on the host during unshard (row-parallel c_proj).

Device kernel (per core, all matmuls bf16, fp32 PSUM accumulation):
  1. qkv^T = w^T x^T via TensorE with contraction over C (x^T supplied
     pre-transposed + bf16 by the host).  Q is pre-scaled by 1/sqrt(D) and
     Q/K biases are added on evacuation (V bias is folded into b_proj on
     the host since softmax rows sum to 1).
  2. Per head pair: S^T[k,q] = K^T.T @ Q^T blocks with the two heads'
     K=64 matmuls issued back-to-back at PE row-tiles (0,0)/(64,0) so they
     run concurrently; exp on ScalarE straight out of PSUM with the
     leading causally-dead columns trimmed; diagonal-block masking on DVE.
  3. AV with V augmented by a ones column -> row sums land in PSUM
     partition 64 for free; normalization = reciprocal_approx_fast (DVE,
     straight from PSUM) + gpsimd broadcast + one tensor-tensor multiply.
  4. Output projection computed transposed (out^T[c,t], contraction over
     the head dim) so b_proj lands as a per-partition tensor_scalar add;
     host re-transposes.  QKV/proj tensor work is interleaved into the
     ScalarE-bound attention rounds as PE filler.
"""

import sys

if "/opt/trn_rl_repo" not in sys.path:
    sys.path.insert(0, "/opt/trn_rl_repo")

import numpy as np
import ml_dtypes

import concourse.bass as bass  # noqa: F401  (engine types pulled via nc)
import concourse.mybir as mybir
from concourse import bacc
from concourse.tile import TileContext
from concourse.bass_utils import run_bass_kernel_spmd

BF16 = ml_dtypes.bfloat16

B, T, C = 4, 2048, 768
H, D = 12, 64
NH = 6  # heads per core
P = 128
TC = T // P  # 16 t-chunks of 128
QC = T // 512  # 4 q-chunks of 512
CCH = C // P  # 6 contraction chunks

DT = mybir.dt.bfloat16
F32 = mybir.dt.float32


def build_nc():
    nc = bacc.Bacc()

    xt_d = nc.declare_dram_parameter("xt", [P, CCH, T], DT, isOutput=False)
    wqk_d = nc.declare_dram_parameter("wqk", [P, CCH, 2 * NH * D], DT, isOutput=False)
    bqk_d = nc.declare_dram_parameter("bqk", [P, 2 * NH * D // P], F32, isOutput=False)
    wv_d = nc.declare_dram_parameter("wv", [P, CCH, NH * D], DT, isOutput=False)
    wp_d = nc.declare_dram_parameter(
        "wp", [P, NH * D // P, C // P, P], DT, isOutput=False
    )
    bp_d = nc.declare_dram_parameter("bp", [P, C // P], F32, isOutput=False)
    mask_d = nc.declare_dram_parameter("mask", [P, P], DT, isOutput=False)
    out_d = nc.declare_dram_parameter("out", [P, C // P, T], F32, isOutput=True)

    with TileContext(nc) as tc:
        with (
            tc.tile_pool(name="consts", bufs=1) as consts,
            tc.tile_pool(name="work", bufs=4) as work,
            tc.tile_pool(name="outp", bufs=3) as outp,
            tc.tile_pool(name="ppool", bufs=1) as ppool,
            tc.tile_pool(name="ps_s", bufs=1, space="PSUM") as ps_s,
            tc.tile_pool(name="ps_qkv", bufs=2, space="PSUM") as ps_qkv,
            tc.tile_pool(name="ps_av", bufs=1, space="PSUM") as ps_av,
        ):
            # ---- load inputs ----
            # inputs split across the two HWDGE queues (sync + scalar);
            # round-0 dependencies (xt quarter 0, wqk, wv, bqk) first
            # sync queue: xt quarter 0 first (round-0 moving operand), then wv
            xt_sb = consts.tile([P, CCH, T], DT)
            nc.sync.dma_start(xt_sb[:, :, 0 : T // 4], xt_d[:, :, 0 : T // 4])
            wv_sb = consts.tile([P, CCH, NH * D], DT)
            nc.sync.dma_start(wv_sb[:], wv_d[:])
            # scalar queue: mask (warmup operand) + per-fc wqk slices so the
            # first qk_tiles start ~15us earlier than a monolithic wqk load
            mask_sb = consts.tile([P, P], DT)
            nc.scalar.dma_start(mask_sb[:], mask_d[:])
            wqk_sb = consts.tile([P, CCH, 2 * NH * D], DT)
            for fc in (0, 3):
                nc.scalar.dma_start(
                    wqk_sb[:, :, fc * P : (fc + 1) * P],
                    wqk_d[:, :, fc * P : (fc + 1) * P],
                )
            bqk_sb = consts.tile([P, 2 * NH * D // P], F32)
            nc.scalar.dma_start(bqk_sb[:], bqk_d[:])
            for fc in (1, 4, 2, 5):
                nc.scalar.dma_start(
                    wqk_sb[:, :, fc * P : (fc + 1) * P],
                    wqk_d[:, :, fc * P : (fc + 1) * P],
                )
            for _q in range(1, 4):
                eng = nc.scalar if _q % 2 else nc.sync
                eng.dma_start(
                    xt_sb[:, :, _q * T // 4 : (_q + 1) * T // 4],
                    xt_d[:, :, _q * T // 4 : (_q + 1) * T // 4],
                )
            wp_sb = consts.tile([P, NH * D // P, C // P, P], DT)
            nc.scalar.dma_start(wp_sb[:], wp_d[:])
            bp_sb = consts.tile([P, C // P], F32)
            nc.sync.dma_start(bp_sb[:], bp_d[:])

            # Q^T/K^T as head-pair tiles [128, T]: head 2p in partitions 0:64,
            # head 2p+1 in partitions 64:128 (S-gen uses PE row tiles 0/64)
            qtp = [consts.tile([P, T], DT, name=f"qtp{p}", tag=f"qtp{p}") for p in range(NH // 2)]
            ktp = [consts.tile([P, T], DT, name=f"ktp{p}", tag=f"ktp{p}") for p in range(NH // 2)]
            # V per t-chunk, heads side by side with a ones column: [128, 6, 65]
            vt = [consts.tile([P, NH, D + 1], DT, name=f"vt{t}", tag=f"vt{t}") for t in range(TC)]
            for t in range(TC):
                nc.gpsimd.memset(vt[t][:, :, D : D + 1], 1.0)
            # y^T per head-pair [128, T] bf16
            yt = [consts.tile([P, T], DT, name=f"yt{p}", tag=f"yt{p}") for p in range(NH // 2)]



            # ---- filler units (QKV / V / proj work interleaved into the
            # ScalarE-bound attention rounds to keep the PE busy) ----
            def qk_tile(fc, tcb):
                # feat chunk fc: 0..2 -> Q pair fc, 3..5 -> K pair fc-3
                pq = ps_qkv.tile([P, 512], F32, tag="qkv", name="pq")
                for cc in range(CCH):
                    nc.tensor.matmul(
                        pq[:],
                        wqk_sb[:, cc, fc * P : (fc + 1) * P],
                        xt_sb[:, cc, tcb * 512 : (tcb + 1) * 512],
                        start=(cc == 0),
                        stop=(cc == CCH - 1),
                    )
                dst = qtp[fc] if fc < 3 else ktp[fc - 3]
                nc.vector.tensor_scalar_add(
                    dst[:, tcb * 512 : (tcb + 1) * 512],
                    pq[:],
                    bqk_sb[:, fc : fc + 1],
                )

            def v_chunk(t):
                pv = ps_qkv.tile([P, NH * D], F32, tag="qkv", name="pv")
                for cc in range(CCH):
                    nc.tensor.matmul(
                        pv[:],
                        xt_sb[:, cc, t * P : (t + 1) * P],
                        wv_sb[:, cc, :],
                        start=(cc == 0),
                        stop=(cc == CCH - 1),
                    )
                nc.vector.tensor_copy(
                    vt[t][:, :, 0:D],
                    pv[:].rearrange("p (h d) -> p h d", d=D),
                )

            def proj(cc, qc):
                # out^T[c-chunk cc, t-range qc]: contract over head dim
                pp = ps_qkv.tile([P, 512], F32, tag="qkv", name="pp")
                for hp in range(NH // 2):
                    nc.tensor.matmul(
                        pp[:],
                        wp_sb[:, hp, cc, :],
                        yt[hp][:, qc * 512 : (qc + 1) * 512],
                        start=(hp == 0),
                        stop=(hp == NH // 2 - 1),
                    )
                stg = outp.tile([P, 512], F32)
                nc.vector.tensor_scalar_add(stg[:], pp[:], bp_sb[:, cc : cc + 1])
                nc.sync.dma_start(out_d[:, cc, qc * 512 : (qc + 1) * 512], stg[:])

            # last-round proj is split so the hp 0/1 partials run mid-round
            # and only the hp 2 matmul + finish are left for the tail
            proj_part = {}

            def proj_a(cc, qc):
                pp = ps_qkv.tile([P, 512], F32, tag="qkv", name="ppa")
                for hp in (0, 1):
                    nc.tensor.matmul(
                        pp[:],
                        wp_sb[:, hp, cc, :],
                        yt[hp][:, qc * 512 : (qc + 1) * 512],
                        start=(hp == 0),
                        stop=(hp == 1),
                    )
                part = ppool.tile([P, 512], F32, tag=f"part{cc}", name="part")
                proj_part[cc] = part
                nc.vector.tensor_scalar_add(part[:], pp[:], bp_sb[:, cc : cc + 1])

            def proj_b(cc, qc):
                pp = ps_qkv.tile([P, 512], F32, tag="qkv", name="ppb")
                nc.tensor.matmul(
                    pp[:],
                    wp_sb[:, 2, cc, :],
                    yt[2][:, qc * 512 : (qc + 1) * 512],
                    start=True,
                    stop=True,
                )
                stg = outp.tile([P, 512], F32)
                nc.vector.tensor_add(stg[:], pp[:], proj_part[cc][:])
                nc.sync.dma_start(out_d[:, cc, qc * 512 : (qc + 1) * 512], stg[:])

            # ---- attention: flat software pipeline over (qc, hp, g) with the
            # AV stage lagging one group behind the S+exp stage, so the PE
            # always has ready S-pair work ahead of the exp-gated AV matmuls
            av_tiles = {}

            def s_exp_stage(qc, hp, g):
                sps, sexp = [], []
                for hi in (0, 1):
                    sps.append(ps_s.tile([P, 1024], F32, tag=f"s{hi}", name=f"sps{hi}"))
                # S pair: interleave the two heads' K=64 matmuls so the
                # PE row-tiles (0,0)/(64,0) execute them concurrently
                for sub in range(2):
                    j = 2 * g + sub
                    m = max(0, (j - 4 * qc) * P)  # causal trim offset
                    # keep sub 1 un-trimmed so the exp input range
                    # [m0:1024] is fully written (extra cols never read)
                    mw = m if sub == 0 else 0
                    for hi in (0, 1):
                        b0 = 64 * hi
                        nc.tensor.matmul(
                            sps[hi][:, sub * 512 + mw : (sub + 1) * 512],
                            ktp[hp][b0 : b0 + 64, j * P : (j + 1) * P],
                            qtp[hp][b0 : b0 + 64, qc * 512 + mw : (qc + 1) * 512],
                            start=True,
                            stop=True,
                        )
                m0 = max(0, (2 * g - 4 * qc) * P)  # leading dead cols
                for hi in (0, 1):
                    se = work.tile([P, 1024], DT, tag="sexp", name="sexp")
                    sexp.append(se)
                    nc.scalar.activation(
                        se[:, m0:1024],
                        sps[hi][:, m0:1024],
                        mybir.ActivationFunctionType.Exp,
                    )
                return sexp

            def av_stage(qc, hp, g, sexp):
                nj = 4 * (qc + 1)
                if g == 0:
                    av_tiles[(qc, hp)] = [
                        ps_av.tile([65, 512], F32, tag=f"av{hi}", name=f"av{hi}")
                        for hi in (0, 1)
                    ]
                av = av_tiles[(qc, hp)]
                for hi in (0, 1):
                    for sub in range(2):
                        j = 2 * g + sub
                        m = max(0, (j - 4 * qc) * P)
                        if j - 4 * qc >= 0:  # diagonal block: mask
                            nc.vector.tensor_mul(
                                sexp[hi][:, sub * 512 + m : sub * 512 + m + P],
                                sexp[hi][:, sub * 512 + m : sub * 512 + m + P],
                                mask_sb[:],
                            )
                        nc.tensor.matmul(
                            av[hi][:, m:512],
                            vt[j][:, 2 * hp + hi, :],
                            sexp[hi][:, sub * 512 + m : (sub + 1) * 512],
                            start=(j == 0),
                            stop=(j == nj - 1),
                            skip_group_check=True,
                        )
                if 2 * g + 1 == nj - 1:  # last group: normalize
                    for hi in (0, 1):
                        b0 = 64 * hi
                        # two copies evacuate the av PSUM slot quickly; the
                        # rest of the norm chain runs from SBUF.  The sums
                        # row is copied to partition 0: the custom-DVE
                        # reciprocal misbehaves on a partition-64 source.
                        yc = work.tile([64, 512], F32, tag="yc", name="yc")
                        nc.vector.tensor_copy(yc[:], av[hi][0:64, :])
                        sums = work.tile([1, 512], F32, tag="sums", name="sums")
                        nc.vector.tensor_copy(sums[:], av[hi][64:65, :])
                        inv = work.tile([1, 512], F32, tag="inv", name="inv")
                        nc.vector.reciprocal_approx_fast(inv[:], sums[:])
                        invb = work.tile([64, 512], F32, tag="invb", name="invb")
                        nc.gpsimd.partition_broadcast(invb[:], inv[:])
                        nc.vector.tensor_mul(
                            yt[hp][b0 : b0 + 64, qc * 512 : (qc + 1) * 512],
                            yc[:],
                            invb[:],
                        )

            # ---- warm the PE HAM clock gate during the input DMA wait ----
            for w in range(24):
                wps = ps_qkv.tile([P, P], F32, tag="qkv", name="warm")
                nc.tensor.matmul(wps[:], mask_sb[:], mask_sb[:], start=True, stop=True)

            # ---- schedule: flat software pipeline over (qc, hp, g) with a
            # deadline-driven filler queue.  Q tiles are due at their round's
            # start; K/V tiles are lazily due at their first (diagonal) use
            # inside their own round; proj has no deadline.  This levels the
            # PE filler load across the growing causal rounds.
            flat = [
                (qc, hp, g)
                for qc in range(QC)
                for hp in range(3)
                for g in range(2 * (qc + 1))
            ]
            fidx = {key: i for i, key in enumerate(flat)}
            END = len(flat) + 1
            sched = []  # (deadline flat-index, closure), FIFO within pushes

            def push(deadline, fn):
                sched.append([deadline, fn])

            def emit_due(i):
                rest = []
                for item in sched:
                    if item[0] <= i:
                        item[1]()
                    else:
                        rest.append(item)
                sched[:] = rest

            # round 0 critical prefix
            qk_tile(0, 0)
            qk_tile(3, 0)
            for fc in (1, 2):
                push(fidx[(0, fc, 0)], lambda fc=fc: qk_tile(fc, 0))
                push(fidx[(0, fc, 0)], lambda fc=fc: qk_tile(3 + fc, 0))
            for t in range(4):
                push(fidx[(0, 0, t // 2)] + 1, lambda t=t: v_chunk(t))

            prev = None
            cur_qc = -1
            periods_left = 0
            for i, (qc, hp, g) in enumerate(flat):
                if qc != cur_qc:
                    cur_qc = qc
                    periods_left = 6 * (qc + 1)
                    if qc + 1 < QC:
                        r = qc + 1
                        for fc in range(3):
                            push(fidx[(r, fc, 0)], lambda fc=fc, r=r: qk_tile(fc, r))
                        for p in range(3):
                            push(
                                fidx[(r, p, 2 * r)],
                                lambda p=p, r=r: qk_tile(3 + p, r),
                            )
                        for t in range(4 * r, 4 * r + 4):
                            push(
                                fidx[(r, 0, t // 2)] + 1,
                                lambda t=t: v_chunk(t),
                            )
                    if qc > 0:
                        for cc in range(C // P):
                            push(END, lambda cc=cc, qc=qc: proj(cc, qc - 1))
                if qc == QC - 1 and hp == 2 and g == 0:
                    # hp 0/1 of the last round are normalized: queue partials
                    for cc in range(C // P):
                        push(END, lambda cc=cc: proj_a(cc, QC - 1))
                emit_due(i)
                sexp = s_exp_stage(qc, hp, g)
                if prev is not None:
                    av_stage(*prev)
                    npop = min(len(sched), 2 if len(sched) > periods_left else 1)
                    for _ in range(npop):
                        sched.pop(0)[1]()
                prev = (qc, hp, g, sexp)
                periods_left -= 1
            av_stage(*prev)
            # keep the PE warm through the final normalization chain
            for w in range(12):
                wps = ps_qkv.tile([P, 512], F32, tag="qkv", name="warm2")
                nc.tensor.matmul(
                    wps[:], mask_sb[:], xt_sb[:, 0, 0:512], start=True, stop=True
                )
            for item in sched:
                item[1]()
            for cc in range(C // P):
                proj_b(cc, QC - 1)

    nc.finalize()
    return nc


def shard_inputs(x, w_attn, b_attn, w_proj, b_proj):
    """Host-side prep: slice per core, transpose x, cast to bf16."""
    scale = 1.0 / np.sqrt(D)
    tril = np.tril(np.ones((P, P), np.float32))
    # mask[k_local, q_local] = 1 where k <= q
    mask = tril.T.astype(BF16)
    in_maps = []
    for core in range(8):
        b, half = divmod(core, 2)
        h0 = half * NH
        cq = slice(h0 * D, (h0 + NH) * D)
        ck = slice(C + h0 * D, C + (h0 + NH) * D)
        cv = slice(2 * C + h0 * D, 2 * C + (h0 + NH) * D)
        wq = (w_attn[:, cq] * scale).astype(BF16)
        wk = w_attn[:, ck].astype(BF16)
        wqk = np.concatenate([wq, wk], axis=1)  # [C, 768]
        bqk = np.concatenate([(b_attn[cq] * scale), b_attn[ck]], axis=0).astype(
            np.float32
        )
        bqk_col = np.ascontiguousarray(bqk.reshape(2 * NH * D // P, P).T)
        wv = w_attn[:, cv].astype(BF16)
        bv = b_attn[cv].astype(np.float32)
        wp = w_proj[h0 * D : (h0 + NH) * D, :].astype(np.float32)
        # V bias rides through softmax (rows sum to 1): fold into proj bias
        bp_eff = bv @ wp + (b_proj if half == 0 else np.zeros_like(b_proj))
        bp_col = np.ascontiguousarray(bp_eff.reshape(C // P, P).T.astype(np.float32))
        xt = np.ascontiguousarray(x[b].T).astype(BF16)  # [C, T]
        in_maps.append(
            {
                "xt": np.ascontiguousarray(
                    xt.reshape(CCH, P, T).transpose(1, 0, 2)
                ),
                "wqk": np.ascontiguousarray(
                    wqk.reshape(CCH, P, 2 * NH * D).transpose(1, 0, 2)
                ),
                "bqk": bqk_col,
                "wv": np.ascontiguousarray(
                    wv.reshape(CCH, P, NH * D).transpose(1, 0, 2)
                ),
                "wp": np.ascontiguousarray(
                    wp.astype(BF16).reshape(NH * D // P, P, C // P, P).transpose(1, 0, 2, 3)
                ),
                "bp": bp_col,
                "mask": mask,
            }
        )
    return in_maps


_NC = None


def _get_nc():
    global _NC
    if _NC is None:
        _NC = build_nc()
    return _NC


def run_sharded(in_maps, trace=False, **kw):
    nc = _get_nc()
    return run_bass_kernel_spmd(nc, in_maps, core_ids=list(range(8)), trace=trace, **kw)


def gather(results):
    out = np.zeros((B, T, C), np.float32)
    for core in range(8):
        b = core // 2
        # out^T [P, C//P, T]: C index = cc*128 + p
        ot = results[core]["out"]
        out[b] += ot.transpose(1, 0, 2).reshape(C, T).T
    return out


def kernel(x, w_attn, b_attn, w_proj, b_proj):
    x = np.asarray(x, np.float32)
    w_attn = np.asarray(w_attn, np.float32)
    b_attn = np.asarray(b_attn, np.float32)
    w_proj = np.asarray(w_proj, np.float32)
    b_proj = np.asarray(b_proj, np.float32)
    in_maps = shard_inputs(x, w_attn, b_attn, w_proj, b_proj)
    res = run_sharded(in_maps, trace=False)
    return gather(res.results)


# revision 26
# speedup vs baseline: 1.8140x; 1.0161x over previous
"""Causal self-attention (GPT-2 small block shape: B=4, T=2048, C=768, H=12, D=64)
on 8 TRN2 NeuronCores.

Sharding: core i handles batch b = i//2 and head-half = i%2 (6 heads each).
No cross-core collectives; the two half-head partial output projections per
batch are summed on the host during unshard (row-parallel c_proj).

Device kernel (per core, all matmuls bf16, fp32 PSUM accumulation):
  1. qkv^T = w^T x^T via TensorE with contraction over C (x^T supplied
     pre-transposed + bf16 by the host).  Q is pre-scaled by 1/sqrt(D) and
     Q/K biases are added on evacuation (V bias is folded into b_proj on
     the host since softmax rows sum to 1).
  2. Per head pair: S^T[k,q] = K^T.T @ Q^T blocks with the two heads'
     K=64 matmuls issued back-to-back at PE row-tiles (0,0)/(64,0) so they
     run concurrently; exp on ScalarE straight out of PSUM with the
     leading causally-dead columns trimmed; diagonal-block masking on DVE.
  3. AV with V augmented by a ones column -> row sums land in PSUM
     partition 64 for free; normalization = reciprocal_approx_fast (DVE,
     straight from PSUM) + gpsimd broadcast + one tensor-tensor multiply.
  4. Output projection computed transposed (out^T[c,t], contraction over
     the head dim) so b_proj lands as a per-partition tensor_scalar add;
     host re-transposes.  QKV/proj tensor work is interleaved into the
     ScalarE-bound attention rounds as PE filler.
"""

import sys

if "/opt/trn_rl_repo" not in sys.path:
    sys.path.insert(0, "/opt/trn_rl_repo")

import numpy as np
import ml_dtypes

import concourse.bass as bass  # noqa: F401  (engine types pulled via nc)
import concourse.mybir as mybir
from concourse import bacc
from concourse.tile import TileContext
from concourse.bass_utils import run_bass_kernel_spmd

BF16 = ml_dtypes.bfloat16

B, T, C = 4, 2048, 768
H, D = 12, 64
NH = 6  # heads per core
P = 128
TC = T // P  # 16 t-chunks of 128
QC = T // 512  # 4 q-chunks of 512
CCH = C // P  # 6 contraction chunks

DT = mybir.dt.bfloat16
F32 = mybir.dt.float32


def build_nc():
    nc = bacc.Bacc()

    xt_d = nc.declare_dram_parameter("xt", [P, CCH, T], DT, isOutput=False)
    wqk_d = nc.declare_dram_parameter("wqk", [P, CCH, 2 * NH * D], DT, isOutput=False)
    bqk_d = nc.declare_dram_parameter("bqk", [P, 2 * NH * D // P], F32, isOutput=False)
    wv_d = nc.declare_dram_parameter("wv", [P, CCH, NH * D], DT, isOutput=False)
    wp_d = nc.declare_dram_parameter(
        "wp", [P, NH * D // P, C // P, P], DT, isOutput=False
    )
    bp_d = nc.declare_dram_parameter("bp", [P, C // P], F32, isOutput=False)
    mask_d = nc.declare_dram_parameter("mask", [P, P], DT, isOutput=False)
    out_d = nc.declare_dram_parameter("out", [P, C // P, T], F32, isOutput=True)

    with TileContext(nc) as tc:
        with (
            tc.tile_pool(name="consts", bufs=1) as consts,
            tc.tile_pool(name="work", bufs=4) as work,
            tc.tile_pool(name="outp", bufs=3) as outp,
            tc.tile_pool(name="ppool", bufs=1) as ppool,
            tc.tile_pool(name="ps_s", bufs=1, space="PSUM") as ps_s,
            tc.tile_pool(name="ps_qkv", bufs=2, space="PSUM") as ps_qkv,
            tc.tile_pool(name="ps_av", bufs=1, space="PSUM") as ps_av,
        ):
            # ---- load inputs ----
            # inputs split across the two HWDGE queues (sync + scalar);
            # round-0 dependencies (xt quarter 0, wqk, wv, bqk) first
            # sync queue: xt quarter 0 first (round-0 moving operand), then wv
            xt_sb = consts.tile([P, CCH, T], DT)
            nc.sync.dma_start(xt_sb[:, :, 0 : T // 4], xt_d[:, :, 0 : T // 4])
            wv_sb = consts.tile([P, CCH, NH * D], DT)
            nc.sync.dma_start(wv_sb[:], wv_d[:])
            # scalar queue: mask (warmup operand) + per-fc wqk slices so the
            # first qk_tiles start ~15us earlier than a monolithic wqk load
            mask_sb = consts.tile([P, P], DT)
            nc.scalar.dma_start(mask_sb[:], mask_d[:])
            wqk_sb = consts.tile([P, CCH, 2 * NH * D], DT)
            for fc in (0, 3):
                nc.scalar.dma_start(
                    wqk_sb[:, :, fc * P : (fc + 1) * P],
                    wqk_d[:, :, fc * P : (fc + 1) * P],
                )
            bqk_sb = consts.tile([P, 2 * NH * D // P], F32)
            nc.scalar.dma_start(bqk_sb[:], bqk_d[:])
            for fc in (1, 4, 2, 5):
                nc.scalar.dma_start(
                    wqk_sb[:, :, fc * P : (fc + 1) * P],
                    wqk_d[:, :, fc * P : (fc + 1) * P],
                )
            for _q in range(1, 4):
                eng = nc.scalar if _q % 2 else nc.sync
                eng.dma_start(
                    xt_sb[:, :, _q * T // 4 : (_q + 1) * T // 4],
                    xt_d[:, :, _q * T // 4 : (_q + 1) * T // 4],
                )
            wp_sb = consts.tile([P, NH * D // P, C // P, P], DT)
            nc.scalar.dma_start(wp_sb[:], wp_d[:])
            bp_sb = consts.tile([P, C // P], F32)
            nc.sync.dma_start(bp_sb[:], bp_d[:])

            # Q^T/K^T as head-pair tiles [128, T]: head 2p in partitions 0:64,
            # head 2p+1 in partitions 64:128 (S-gen uses PE row tiles 0/64)
            qtp = [consts.tile([P, T], DT, name=f"qtp{p}", tag=f"qtp{p}") for p in range(NH // 2)]
            ktp = [consts.tile([P, T], DT, name=f"ktp{p}", tag=f"ktp{p}") for p in range(NH // 2)]
            # V per t-chunk, heads side by side with a ones column: [128, 6, 65]
            vt = [consts.tile([P, NH, D + 1], DT, name=f"vt{t}", tag=f"vt{t}") for t in range(TC)]
            for t in range(TC):
                nc.gpsimd.memset(vt[t][:, :, D : D + 1], 1.0)
            # y^T per head-pair [128, T] bf16
            yt = [consts.tile([P, T], DT, name=f"yt{p}", tag=f"yt{p}") for p in range(NH // 2)]



            # ---- filler units (QKV / V / proj work interleaved into the
            # ScalarE-bound attention rounds to keep the PE busy) ----
            def qk_tile(fc, tcb):
                # feat chunk fc: 0..2 -> Q pair fc, 3..5 -> K pair fc-3
                pq = ps_qkv.tile([P, 512], F32, tag="qkv", name="pq")
                for cc in range(CCH):
                    nc.tensor.matmul(
                        pq[:],
                        wqk_sb[:, cc, fc * P : (fc + 1) * P],
                        xt_sb[:, cc, tcb * 512 : (tcb + 1) * 512],
                        start=(cc == 0),
                        stop=(cc == CCH - 1),
                    )
                dst = qtp[fc] if fc < 3 else ktp[fc - 3]
                nc.vector.tensor_scalar_add(
                    dst[:, tcb * 512 : (tcb + 1) * 512],
                    pq[:],
                    bqk_sb[:, fc : fc + 1],
                )

            def v_chunk(t):
                pv = ps_qkv.tile([P, NH * D], F32, tag="qkv", name="pv")
                for cc in range(CCH):
                    nc.tensor.matmul(
                        pv[:],
                        xt_sb[:, cc, t * P : (t + 1) * P],
                        wv_sb[:, cc, :],
                        start=(cc == 0),
                        stop=(cc == CCH - 1),
                    )
                nc.vector.tensor_copy(
                    vt[t][:, :, 0:D],
                    pv[:].rearrange("p (h d) -> p h d", d=D),
                )

            def proj(cc, qc):
                # out^T[c-chunk cc, t-range qc]: contract over head dim
                pp = ps_qkv.tile([P, 512], F32, tag="qkv", name="pp")
                for hp in range(NH // 2):
                    nc.tensor.matmul(
                        pp[:],
                        wp_sb[:, hp, cc, :],
                        yt[hp][:, qc * 512 : (qc + 1) * 512],
                        start=(hp == 0),
                        stop=(hp == NH // 2 - 1),
                    )
                stg = outp.tile([P, 512], F32)
                nc.vector.tensor_scalar_add(stg[:], pp[:], bp_sb[:, cc : cc + 1])
                nc.sync.dma_start(out_d[:, cc, qc * 512 : (qc + 1) * 512], stg[:])

            # last-round proj is split so the hp 0/1 partials run mid-round
            # and only the hp 2 matmul + finish are left for the tail
            proj_part = {}

            def proj_a(cc, qc):
                pp = ps_qkv.tile([P, 512], F32, tag="qkv", name="ppa")
                for hp in (0, 1):
                    nc.tensor.matmul(
                        pp[:],
                        wp_sb[:, hp, cc, :],
                        yt[hp][:, qc * 512 : (qc + 1) * 512],
                        start=(hp == 0),
                        stop=(hp == 1),
                    )
                part = ppool.tile([P, 512], F32, tag=f"part{cc}", name="part")
                proj_part[cc] = part
                nc.vector.tensor_scalar_add(part[:], pp[:], bp_sb[:, cc : cc + 1])

            def proj_b(cc, qc):
                pp = ps_qkv.tile([P, 512], F32, tag="qkv", name="ppb")
                nc.tensor.matmul(
                    pp[:],
                    wp_sb[:, 2, cc, :],
                    yt[2][:, qc * 512 : (qc + 1) * 512],
                    start=True,
                    stop=True,
                )
                stg = outp.tile([P, 512], F32)
                nc.vector.tensor_add(stg[:], pp[:], proj_part[cc][:])
                nc.sync.dma_start(out_d[:, cc, qc * 512 : (qc + 1) * 512], stg[:])

            # ---- attention: flat software pipeline over (qc, hp, g) with the
            # AV stage lagging one group behind the S+exp stage, so the PE
            # always has ready S-pair work ahead of the exp-gated AV matmuls
            av_tiles = {}

            def s_exp_stage(qc, hp, g):
                sps, sexp = [], []
                for hi in (0, 1):
                    sps.append(ps_s.tile([P, 1024], F32, tag=f"s{hi}", name=f"sps{hi}"))
                # S pair: interleave the two heads' K=64 matmuls so the
                # PE row-tiles (0,0)/(64,0) execute them concurrently
                for sub in range(2):
                    j = 2 * g + sub
                    m = max(0, (j - 4 * qc) * P)  # causal trim offset
                    # keep sub 1 un-trimmed so the exp input range
                    # [m0:1024] is fully written (extra cols never read)
                    mw = m if sub == 0 else 0
                    for hi in (0, 1):
                        b0 = 64 * hi
                        nc.tensor.matmul(
                            sps[hi][:, sub * 512 + mw : (sub + 1) * 512],
                            ktp[hp][b0 : b0 + 64, j * P : (j + 1) * P],
                            qtp[hp][b0 : b0 + 64, qc * 512 + mw : (qc + 1) * 512],
                            start=True,
                            stop=True,
                        )
                m0 = max(0, (2 * g - 4 * qc) * P)  # leading dead cols
                for hi in (0, 1):
                    se = work.tile([P, 1024], DT, tag="sexp", name="sexp")
                    sexp.append(se)
                    nc.scalar.activation(
                        se[:, m0:1024],
                        sps[hi][:, m0:1024],
                        mybir.ActivationFunctionType.Exp,
                    )
                return sexp

            def av_stage(qc, hp, g, sexp):
                nj = 4 * (qc + 1)
                if g == 0:
                    av_tiles[(qc, hp)] = [
                        ps_av.tile([65, 512], F32, tag=f"av{hi}", name=f"av{hi}")
                        for hi in (0, 1)
                    ]
                av = av_tiles[(qc, hp)]
                for hi in (0, 1):
                    for sub in range(2):
                        j = 2 * g + sub
                        m = max(0, (j - 4 * qc) * P)
                        if j - 4 * qc >= 0:  # diagonal block: mask
                            nc.vector.tensor_mul(
                                sexp[hi][:, sub * 512 + m : sub * 512 + m + P],
                                sexp[hi][:, sub * 512 + m : sub * 512 + m + P],
                                mask_sb[:],
                            )
                        nc.tensor.matmul(
                            av[hi][:, m:512],
                            vt[j][:, 2 * hp + hi, :],
                            sexp[hi][:, sub * 512 + m : (sub + 1) * 512],
                            start=(j == 0),
                            stop=(j == nj - 1),
                            skip_group_check=True,
                        )
                if 2 * g + 1 == nj - 1:  # last group: normalize
                    for hi in (0, 1):
                        b0 = 64 * hi
                        # sums row copied to partition 0: the custom-DVE
                        # reciprocal misbehaves on a partition-64 source
                        sums = work.tile([1, 512], F32, tag="sums", name="sums")
                        nc.vector.tensor_copy(sums[:], av[hi][64:65, :])
                        inv = work.tile([1, 512], F32, tag="inv", name="inv")
                        nc.vector.reciprocal_approx_fast(inv[:], sums[:])
                        invb = work.tile([64, 512], F32, tag="invb", name="invb")
                        nc.gpsimd.partition_broadcast(invb[:], inv[:])
                        nc.vector.tensor_mul(
                            yt[hp][b0 : b0 + 64, qc * 512 : (qc + 1) * 512],
                            av[hi][0:64, :],
                            invb[:],
                        )

            # ---- warm the PE HAM clock gate during the input DMA wait ----
            for w in range(14):
                wps = ps_qkv.tile([P, P], F32, tag="qkv", name="warm")
                nc.tensor.matmul(wps[:], mask_sb[:], mask_sb[:], start=True, stop=True)

            # ---- schedule: flat software pipeline over (qc, hp, g) with a
            # deadline-driven filler queue.  Q tiles are due at their round's
            # start; K/V tiles are lazily due at their first (diagonal) use
            # inside their own round; proj has no deadline.  This levels the
            # PE filler load across the growing causal rounds.
            flat = [
                (qc, hp, g)
                for qc in range(QC)
                for hp in range(3)
                for g in range(2 * (qc + 1))
            ]
            fidx = {key: i for i, key in enumerate(flat)}
            END = len(flat) + 1
            sched = []  # (deadline flat-index, closure), FIFO within pushes

            def push(deadline, fn):
                sched.append([deadline, fn])

            def emit_due(i):
                rest = []
                for item in sched:
                    if item[0] <= i:
                        item[1]()
                    else:
                        rest.append(item)
                sched[:] = rest

            # round 0 critical prefix
            qk_tile(0, 0)
            qk_tile(3, 0)
            for fc in (1, 2):
                push(fidx[(0, fc, 0)], lambda fc=fc: qk_tile(fc, 0))
                push(fidx[(0, fc, 0)], lambda fc=fc: qk_tile(3 + fc, 0))
            for t in range(4):
                push(fidx[(0, 0, t // 2)] + 1, lambda t=t: v_chunk(t))

            prev = None
            cur_qc = -1
            periods_left = 0
            for i, (qc, hp, g) in enumerate(flat):
                if qc != cur_qc:
                    cur_qc = qc
                    periods_left = 6 * (qc + 1)
                    if qc + 1 < QC:
                        r = qc + 1
                        for fc in range(3):
                            push(fidx[(r, fc, 0)], lambda fc=fc, r=r: qk_tile(fc, r))
                        for p in range(3):
                            push(
                                fidx[(r, p, 2 * r)],
                                lambda p=p, r=r: qk_tile(3 + p, r),
                            )
                        for t in range(4 * r, 4 * r + 4):
                            push(
                                fidx[(r, 0, t // 2)] + 1,
                                lambda t=t: v_chunk(t),
                            )
                    if qc > 0:
                        for cc in range(C // P):
                            push(END, lambda cc=cc, qc=qc: proj(cc, qc - 1))
                if qc == QC - 1 and hp == 2 and g == 0:
                    # hp 0/1 of the last round are normalized: queue partials
                    for cc in range(C // P):
                        push(END, lambda cc=cc: proj_a(cc, QC - 1))
                emit_due(i)
                sexp = s_exp_stage(qc, hp, g)
                if prev is not None:
                    av_stage(*prev)
                    # pop earliest-deadline first; no-deadline (proj) units
                    # drain only when nothing dated is pending, deferring
                    # them into the ScalarE-paced late rounds
                    round_end = fidx.get((qc + 1, 0, 0), END)
                    dated = sum(1 for it in sched if it[0] < round_end)
                    npop = min(len(sched), 2 if dated >= periods_left else 1)
                    for _ in range(npop):
                        k = min(range(len(sched)), key=lambda j: (sched[j][0], j))
                        sched.pop(k)[1]()
                prev = (qc, hp, g, sexp)
                periods_left -= 1
            av_stage(*prev)
            # keep the PE warm through the final normalization chain
            for w in range(12):
                wps = ps_qkv.tile([P, 512], F32, tag="qkv", name="warm2")
                nc.tensor.matmul(
                    wps[:], mask_sb[:], xt_sb[:, 0, 0:512], start=True, stop=True
                )
            for item in sched:
                item[1]()
            for cc in range(C // P):
                proj_b(cc, QC - 1)

    nc.finalize()
    return nc


def shard_inputs(x, w_attn, b_attn, w_proj, b_proj):
    """Host-side prep: slice per core, transpose x, cast to bf16."""
    scale = 1.0 / np.sqrt(D)
    tril = np.tril(np.ones((P, P), np.float32))
    # mask[k_local, q_local] = 1 where k <= q
    mask = tril.T.astype(BF16)
    in_maps = []
    for core in range(8):
        b, half = divmod(core, 2)
        h0 = half * NH
        cq = slice(h0 * D, (h0 + NH) * D)
        ck = slice(C + h0 * D, C + (h0 + NH) * D)
        cv = slice(2 * C + h0 * D, 2 * C + (h0 + NH) * D)
        wq = (w_attn[:, cq] * scale).astype(BF16)
        wk = w_attn[:, ck].astype(BF16)
        wqk = np.concatenate([wq, wk], axis=1)  # [C, 768]
        bqk = np.concatenate([(b_attn[cq] * scale), b_attn[ck]], axis=0).astype(
            np.float32
        )
        bqk_col = np.ascontiguousarray(bqk.reshape(2 * NH * D // P, P).T)
        wv = w_attn[:, cv].astype(BF16)
        bv = b_attn[cv].astype(np.float32)
        wp = w_proj[h0 * D : (h0 + NH) * D, :].astype(np.float32)
        # V bias rides through softmax (rows sum to 1): fold into proj bias
        bp_eff = bv @ wp + (b_proj if half == 0 else np.zeros_like(b_proj))
        bp_col = np.ascontiguousarray(bp_eff.reshape(C // P, P).T.astype(np.float32))
        xt = np.ascontiguousarray(x[b].T).astype(BF16)  # [C, T]
        in_maps.append(
            {
                "xt": np.ascontiguousarray(
                    xt.reshape(CCH, P, T).transpose(1, 0, 2)
                ),
                "wqk": np.ascontiguousarray(
                    wqk.reshape(CCH, P, 2 * NH * D).transpose(1, 0, 2)
                ),
                "bqk": bqk_col,
                "wv": np.ascontiguousarray(
                    wv.reshape(CCH, P, NH * D).transpose(1, 0, 2)
                ),
                "wp": np.ascontiguousarray(
                    wp.astype(BF16).reshape(NH * D // P, P, C // P, P).transpose(1, 0, 2, 3)
                ),
                "bp": bp_col,
                "mask": mask,
            }
        )
    return in_maps


_NC = None


def _get_nc():
    global _NC
    if _NC is None:
        _NC = build_nc()
    return _NC


def run_sharded(in_maps, trace=False, **kw):
    nc = _get_nc()
    return run_bass_kernel_spmd(nc, in_maps, core_ids=list(range(8)), trace=trace, **kw)


def gather(results):
    out = np.zeros((B, T, C), np.float32)
    for core in range(8):
        b = core // 2
        # out^T [P, C//P, T]: C index = cc*128 + p
        ot = results[core]["out"]
        out[b] += ot.transpose(1, 0, 2).reshape(C, T).T
    return out


def kernel(x, w_attn, b_attn, w_proj, b_proj):
    x = np.asarray(x, np.float32)
    w_attn = np.asarray(w_attn, np.float32)
    b_attn = np.asarray(b_attn, np.float32)
    w_proj = np.asarray(w_proj, np.float32)
    b_proj = np.asarray(b_proj, np.float32)
    in_maps = shard_inputs(x, w_attn, b_attn, w_proj, b_proj)
    res = run_sharded(in_maps, trace=False)
    return gather(res.results)
